# revision 3
# baseline (speedup 1.0000x reference)
"""Multi-head attention (B=2, S=2048, H=1024, NH=16) on 8 TRN2 NeuronCores.

Sharding: fully data/tensor parallel, no collectives. Core c = (b, hg) with
b = c // 4 (batch), hg = c % 4 (head group of 4 heads = 256 of the 1024
projection output dims).

v2 design (vs v1 baseline):
  - No PE transposes at all:
      * q/k projections produce qT/kT [256, S] (dims on partitions) as before
        (W stationary, xT moving).
      * v projection runs the OTHER orientation (x chunk stationary, WvT
        moving) producing v directly as [k, d] — the layout ctx needs.
      * the context is written out UNNORMALIZED as ctxT [65 rows/head, S]
        (row 64 = softmax denominator); the host does divide + transpose
        in fp32 (exact, off the critical path).
  - Single software-pipelined instruction stream: scores rounds are paced
    into the projection phase at the ACT engine's consumption rate, keeping
    both PE and ACT continuously busy (PE p-state stays at max clock).
  - PSUM: never more than 8 banks: psS [128,1024]x2 spans the whole kernel;
    psA (4 banks) / psV (4) / psC (4) are scoped sequentially.
"""

import functools
import sys

if "/opt/trn_rl_repo" not in sys.path:
    sys.path.insert(0, "/opt/trn_rl_repo")

import numpy as np

B, S, H = 2, 2048, 1024
NH, HD = 16, 64
NCORES = 8
GROUPS = 4                # head groups (cores per batch)
DPG = H // GROUPS         # projection dims per core = 256
HPG = DPG // HD           # heads per core = 4
P = 128                   # SBUF partitions
NHC = H // P              # contraction chunks per projection = 8
QB = 512                  # q block (matmul moving free dim)
NQB = S // QB             # 4
NKC = S // P              # k chunks = 16
VA_W = HD + 1             # 64 v dims + ones col (softmax denominator)
VA_PAD = 128              # va slot width (padded: FWL + zero pad rows)

PACE = 2390               # PE cols between scores rounds ~= ACT round cost


@functools.lru_cache(maxsize=1)
def _build():
    import concourse.bacc as bacc
    import concourse.mybir as mybir
    import concourse.tile as tile

    F32 = mybir.dt.float32
    BF16 = mybir.dt.bfloat16
    Exp = mybir.ActivationFunctionType.Exp
    ADD = mybir.AluOpType.add

    nc = bacc.Bacc()

    xq_d = nc.declare_dram_parameter("xq", [H, S], BF16, isOutput=False)
    xk_d = nc.declare_dram_parameter("xk", [H, S], BF16, isOutput=False)
    xv_d = nc.declare_dram_parameter("xv", [H, S], BF16, isOutput=False)
    wq_d = nc.declare_dram_parameter("wq", [H, DPG], BF16, isOutput=False)
    wk_d = nc.declare_dram_parameter("wk", [H, DPG], BF16, isOutput=False)
    wv_d = nc.declare_dram_parameter("wv", [H, DPG], BF16, isOutput=False)
    bqk_d = nc.declare_dram_parameter("bqk", [P, 4], F32, isOutput=False)
    bvb_d = nc.declare_dram_parameter("bvb", [P, DPG], F32, isOutput=False)
    mk_d = nc.declare_dram_parameter("mk", [P, NKC], F32, isOutput=False)
    out_d = nc.declare_dram_parameter("out", [HPG * VA_W, S], F32, isOutput=True)

    # scores-round emission order (h, pr, kc); pr = pair of q blocks (1024 q)
    rounds = (
        [(h, 0, kc) for h in range(HPG) for kc in range(8)]          # rA
        + [(h, 0, kc) for h in range(HPG) for kc in range(8, 16)]    # rB
        + [(h, 1, kc) for h in range(HPG) for kc in range(16)]       # rC
    )
    NR = len(rounds)  # 128
    ridx = {hpk: r for r, hpk in enumerate(rounds)}
    pq = [None] * NR

    # ctx consumption order: pair-major, kc ascending (PSUM accumulation)
    ctx_order = [
        (h, pr, kc) for pr in range(2) for h in range(HPG) for kc in range(NKC)
    ]

    with tile.TileContext(nc) as tc:
        with (
            tc.tile_pool(name="const", bufs=1) as cpool,
            tc.tile_pool(name="proj_out", bufs=1) as projpool,
            tc.tile_pool(name="xt", bufs=6) as xpool,
            tc.tile_pool(name="xvp", bufs=10) as xvpool,
            tc.tile_pool(name="pexp", bufs=44) as ppool,
        ):
            wk_sb = cpool.tile([P, NHC * DPG], BF16)
            wq_sb = cpool.tile([P, NHC * DPG], BF16)
            wv_sb = cpool.tile([P, NHC * DPG], BF16)
            bqk_sb = cpool.tile([P, 4], F32)
            bvb_sb = cpool.tile([P, DPG], F32)
            mk_sb = cpool.tile([P, NKC], F32)

            qT0 = projpool.tile([P, S], BF16)
            qT1 = projpool.tile([P, S], BF16)
            kT0 = projpool.tile([P, S], BF16)
            kT1 = projpool.tile([P, S], BF16)
            va_sb = projpool.tile([P, NKC * HPG * VA_PAD], BF16)

            # only the DMAs the first matmuls need; wq/wv/misc stream in
            # during the kT/qT hc loops
            for hc in range(NHC):
                sl = slice(hc * DPG, (hc + 1) * DPG)
                nc.sync.dma_start(wk_sb[:, sl], wk_d[hc * P : (hc + 1) * P, :])
            nc.sync.dma_start(bqk_sb[:], bqk_d[:])

            # va zero pad + ones cols (DVE idle early anyway)
            nc.vector.memset(va_sb[:], 0.0)
            for sc in range(NKC):
                for h in range(HPG):
                    oc = (sc * HPG + h) * VA_PAD + HD
                    nc.vector.memset(va_sb[:, oc : oc + 1], 1.0)

            # ---- scores round machinery ----
            state = {"emitted": 0, "acc": 0, "ready": 0}

            def scores_round(r, pool):
                h, pr, kc = rounds[r]
                qT_t = qT0 if h < 2 else qT1
                kT_t = kT0 if h < 2 else kT1
                rows = slice((h % 2) * HD, (h % 2) * HD + HD)
                p2 = ppool.tile([P, 2 * QB], BF16, tag="p", name=f"p{r}")
                s2 = pool.tile([P, 2 * QB], F32, tag="s2", name=f"s2_{r}", bufs=2)
                for i in range(2):
                    qb = pr * 2 + i
                    nc.tensor.matmul(
                        s2[:, i * QB : (i + 1) * QB],
                        kT_t[rows, kc * P : (kc + 1) * P],
                        qT_t[rows, qb * QB : (qb + 1) * QB],
                        start=True,
                        stop=True,
                    )
                nc.scalar.activation(
                    p2[:], s2[:], Exp, bias=mk_sb[:, kc : kc + 1], scale=0.125
                )
                pq[r] = p2

            def pump(pool, cols):
                state["acc"] += cols
                while state["acc"] >= PACE and state["emitted"] < state["ready"]:
                    scores_round(state["emitted"], pool)
                    state["emitted"] += 1
                    state["acc"] -= PACE

            # ---- projections ----
            def proj_pair(x_d, w_sb, bcol, dst0, dst1, pr, psA, spool,
                          extra_dma=None):
                cols0 = pr * 2 * QB
                pp = [
                    psA.tile([P, QB], F32, tag=f"pp{j}", name=f"pp{j}", bufs=1)
                    for j in range(4)
                ]
                for hc in range(NHC):
                    if extra_dma is not None:
                        extra_dma(hc)
                    xt = xpool.tile([P, 2 * QB], BF16, tag="xt", name="xt")
                    nc.sync.dma_start(
                        xt[:, :QB],
                        x_d[hc * P : (hc + 1) * P, cols0 : cols0 + QB],
                    )
                    nc.sync.dma_start(
                        xt[:, QB:],
                        x_d[hc * P : (hc + 1) * P, cols0 + QB : cols0 + 2 * QB],
                    )
                    st = dict(start=(hc == 0), stop=(hc == NHC - 1))
                    w0 = w_sb[:, hc * DPG : hc * DPG + P]
                    w1 = w_sb[:, hc * DPG + P : (hc + 1) * DPG]
                    nc.tensor.matmul(pp[0][:], w0, xt[:, :QB], **st)
                    nc.tensor.matmul(pp[1][:], w0, xt[:, QB:], **st)
                    nc.tensor.matmul(pp[2][:], w1, xt[:, :QB], **st)
                    nc.tensor.matmul(pp[3][:], w1, xt[:, QB:], **st)
                    if spool is not None:
                        pump(spool, 4 * QB)
                for j in range(4):
                    dst = dst0 if j < 2 else dst1
                    bc = bcol + (0 if j < 2 else 1)
                    qb = pr * 2 + (j % 2)
                    nc.vector.tensor_scalar(
                        dst[:, qb * QB : (qb + 1) * QB], pp[j][:],
                        bqk_sb[:, bc : bc + 1], None, ADD,
                    )

            # v projection, direct [k, d] orientation: x chunk stationary,
            # WvT moving; group g covers kc chunks 4g..4g+3
            def v_group(g, psV, spool):
                cols0 = g * 4 * P
                xvt = []
                for hc in range(NHC):
                    xt = xvpool.tile(
                        [P, 4 * P], BF16, tag="xv", name=f"xv{g}_{hc}"
                    )
                    nc.sync.dma_start(
                        xt[:, : 2 * P],
                        xv_d[hc * P : (hc + 1) * P, cols0 : cols0 + 2 * P],
                    )
                    nc.sync.dma_start(
                        xt[:, 2 * P :],
                        xv_d[hc * P : (hc + 1) * P, cols0 + 2 * P : cols0 + 4 * P],
                    )
                    xvt.append(xt)
                vp = [
                    psV.tile([P, DPG], F32, tag=f"vp{i}", name=f"vp{i}", bufs=1)
                    for i in range(4)
                ]
                for hc in range(NHC):
                    st = dict(start=(hc == 0), stop=(hc == NHC - 1))
                    for i in range(4):
                        nc.tensor.matmul(
                            vp[i][:],
                            xvt[hc][:, i * P : (i + 1) * P],
                            wv_sb[:, hc * DPG : (hc + 1) * DPG],
                            **st,
                        )
                    if spool is not None:
                        pump(spool, 4 * DPG)
                for i in range(4):
                    kc = g * 4 + i
                    for h in range(HPG):
                        off = (kc * HPG + h) * VA_PAD
                        nc.vector.tensor_tensor(
                            va_sb[:, off : off + HD],
                            vp[i][:, h * HD : (h + 1) * HD],
                            bvb_sb[:, h * HD : (h + 1) * HD],
                            ADD,
                        )

            with tc.tile_pool(name="psS", bufs=1, space="PSUM") as psS:
                with tc.tile_pool(name="psA", bufs=1, space="PSUM") as psA:
                    def dma_wq(hc):
                        sl = slice(hc * DPG, (hc + 1) * DPG)
                        nc.sync.dma_start(
                            wq_sb[:, sl], wq_d[hc * P : (hc + 1) * P, :]
                        )

                    def dma_wv(hc):
                        if hc == 0:
                            nc.sync.dma_start(mk_sb[:], mk_d[:])
                            nc.sync.dma_start(bvb_sb[:], bvb_d[:])
                        sl = slice(hc * DPG, (hc + 1) * DPG)
                        nc.sync.dma_start(
                            wv_sb[:, sl], wv_d[hc * P : (hc + 1) * P, :]
                        )

                    proj_pair(xk_d, wk_sb, 2, kT0, kT1, 0, psA, None,
                              extra_dma=dma_wq)
                    proj_pair(xq_d, wq_sb, 0, qT0, qT1, 0, psA, None,
                              extra_dma=dma_wv)
                    state["ready"] = 32
                    proj_pair(xk_d, wk_sb, 2, kT0, kT1, 1, psA, psS)
                    state["ready"] = 64
                    proj_pair(xq_d, wq_sb, 0, qT0, qT1, 1, psA, psS)
                    state["ready"] = NR

                with tc.tile_pool(name="psV", bufs=1, space="PSUM") as psV:
                    for g in range(4):
                        v_group(g, psV, psS)

                # ---- phase 2: remaining scores + streamed ctx ----
                with tc.tile_pool(name="psC", bufs=1, space="PSUM") as psC:
                    from collections import deque

                    cq = deque(ctx_order)
                    ctx_tiles = {}

                    def ctx_unit():
                        h, pr, kc = cq.popleft()
                        r = ridx[(h, pr, kc)]
                        if (h, pr) not in ctx_tiles:
                            ctx_tiles[(h, pr)] = [
                                psC.tile(
                                    [VA_PAD, QB], F32, tag="ctx",
                                    name=f"ctx{h}_{pr}_{i}", bufs=4,
                                )
                                for i in range(2)
                            ]
                        ct = ctx_tiles[(h, pr)]
                        off = (kc * HPG + h) * VA_PAD
                        for i in range(2):
                            nc.tensor.matmul(
                                ct[i][:],
                                va_sb[:, off : off + VA_PAD],
                                pq[r][:, i * QB : (i + 1) * QB],
                                start=(kc == 0),
                                stop=(kc == NKC - 1),
                            )
                        pq[r] = None
                        if kc == NKC - 1:
                            ct2 = ctx_tiles.pop((h, pr))
                            for i in range(2):
                                qb = pr * 2 + i
                                ob = ppool.tile(
                                    [VA_W, QB], F32, tag="ob",
                                    name=f"ob{h}_{pr}_{i}", bufs=4,
                                )
                                nc.vector.tensor_copy(ob[:], ct2[i][:VA_W, :])
                                nc.sync.dma_start(
                                    out_d[
                                        h * VA_W : (h + 1) * VA_W,
                                        qb * QB : (qb + 1) * QB,
                                    ],
                                    ob[:],
                                )

                    E1 = state["emitted"]
                    for r in range(E1, NR):
                        scores_round(r, psS)
                        n = 0
                        while cq and n < 2:
                            if ridx[cq[0]] + 2 <= r + 1:
                                ctx_unit()
                                n += 1
                            else:
                                break
                    while cq:
                        ctx_unit()

    nc.compile()
    return nc


def _in_maps(query, key, value, attention_mask, Wq, bq, Wk, bk, Wv, bv):
    import ml_dtypes

    bf16 = ml_dtypes.bfloat16
    q = np.asarray(query, np.float32)
    k = np.asarray(key, np.float32)
    v = np.asarray(value, np.float32)
    m = np.asarray(attention_mask, np.float32)
    Wq = np.asarray(Wq, np.float32)
    Wk = np.asarray(Wk, np.float32)
    Wv = np.asarray(Wv, np.float32)
    bq = np.asarray(bq, np.float32)
    bk = np.asarray(bk, np.float32)
    bv = np.asarray(bv, np.float32)

    xT = [
        (
            np.ascontiguousarray(q[b].T).astype(bf16),
            np.ascontiguousarray(k[b].T).astype(bf16),
            np.ascontiguousarray(v[b].T).astype(bf16),
        )
        for b in range(B)
    ]
    maps = []
    for c in range(NCORES):
        b, hg = divmod(c, GROUPS)
        hs = hg * DPG
        he = hs + DPG
        bqs, bks = bq[hs:he], bk[hs:he]
        bqk = np.stack([bqs[:P], bqs[P:], bks[:P], bks[P:]], axis=1).astype(
            np.float32
        )
        bvb = np.ascontiguousarray(
            np.broadcast_to(bv[hs:he][None, :], (P, DPG)).astype(np.float32)
        )
        maps.append(
            {
                "xq": xT[b][0],
                "xk": xT[b][1],
                "xv": xT[b][2],
                "wq": np.ascontiguousarray(Wq[hs:he, :].T).astype(bf16),
                "wk": np.ascontiguousarray(Wk[hs:he, :].T).astype(bf16),
                "wv": np.ascontiguousarray(Wv[hs:he, :].T).astype(bf16),
                "bqk": np.ascontiguousarray(bqk),
                "bvb": bvb,
                "mk": np.ascontiguousarray(m[b, 0, 0].reshape(NKC, P).T),
            }
        )
    return maps


def kernel(query, key, value, attention_mask, Wq, bq, Wk, bk, Wv, bv):
    from concourse.bass_utils import run_bass_kernel_spmd

    nc = _build()
    maps = _in_maps(
        query, key, value, attention_mask, Wq, bq, Wk, bk, Wv, bv
    )
    res = run_bass_kernel_spmd(nc, maps, core_ids=list(range(NCORES)))
    out = np.empty((B, S, H), np.float32)
    for c in range(NCORES):
        b, hg = divmod(c, GROUPS)
        o = np.asarray(res.results[c]["out"], np.float32)  # [4*65, S]
        for h in range(HPG):
            blk = o[h * VA_W : (h + 1) * VA_W]
            ctx = blk[:HD] / blk[HD : HD + 1]
            out[b, :, hg * DPG + h * HD : hg * DPG + (h + 1) * HD] = ctx.T
    return out


# revision 5
# speedup vs baseline: 1.3239x; 1.3239x over previous
"""Multi-head attention (B=2, S=2048, H=1024, NH=16) on 8 TRN2 NeuronCores.

Sharding: fully data/tensor parallel, no collectives. Core c = (b, hg) with
b = c // 4 (batch), hg = c % 4 (head group of 4 heads = 256 of the 1024
projection output dims).

v3 design:
  - No PE transposes: q/k projections produce qT/kT [256, S] (W stationary,
    xT moving); the v projection uses the opposite orientation (x chunk
    stationary, WvT moving) producing v directly in the [k, d] layout the
    context matmul needs; context is written out UNNORMALIZED as ctxT
    [65 rows/head, S] (row 64 = softmax denominator) and the host does
    divide + transpose in fp32.
  - ACT (exp) is the pacer in the attention phase (1038 ns / 1024-wide exp
    x 128 = 133 us); PE floor is 150 us. The schedule keeps ACT fed from
    ~t=19us: scores rounds are paced into the projection stream, the v
    projection is deferred and used as PE filler between rounds, and ctx
    runs i-split (one 512-col accumulator stream at a time -> 2 PSUM banks)
    interleaved ~3 units per round in the final phase.
  - DMA: weights host-repacked to the SBUF layout and striped; all x tiles
    stream as parallel 128KB transfers across queues (per-queue BW is only
    ~22.5 GB/s and each transfer has ~2us fixed latency, so depth matters).
"""

import functools
import sys

if "/opt/trn_rl_repo" not in sys.path:
    sys.path.insert(0, "/opt/trn_rl_repo")

import numpy as np

B, S, H = 2, 2048, 1024
NH, HD = 16, 64
NCORES = 8
GROUPS = 4                # head groups (cores per batch)
DPG = H // GROUPS         # projection dims per core = 256
HPG = DPG // HD           # heads per core = 4
P = 128                   # SBUF partitions
NHC = H // P              # contraction chunks per projection = 8
QB = 512                  # q block (matmul moving free dim)
NQB = S // QB             # 4
NKC = S // P              # k chunks = 16
VA_W = HD + 1             # 64 v dims + ones col (softmax denominator)
VA_PAD = 128              # va slot width (padded; FWL + zero pad rows)
NVG = 4                   # v projection groups (4 kc chunks each)

PACE = 1480               # extra PE cols between scores rounds (ACT pacing)
CTX_FILL = 1467           # ctx cols budget per round in the final phase
CTX_LAG = 4               # rounds of scores lead required before ctx reads


@functools.lru_cache(maxsize=1)
def _build():
    import concourse.bacc as bacc
    import concourse.mybir as mybir
    import concourse.tile as tile

    F32 = mybir.dt.float32
    BF16 = mybir.dt.bfloat16
    Exp = mybir.ActivationFunctionType.Exp
    ADD = mybir.AluOpType.add

    nc = bacc.Bacc()

    xq_d = nc.declare_dram_parameter("xq", [H, S], BF16, isOutput=False)
    xk_d = nc.declare_dram_parameter("xk", [H, S], BF16, isOutput=False)
    xv_d = nc.declare_dram_parameter("xv", [H, S], BF16, isOutput=False)
    # weights host-repacked to the exact SBUF layout [P, NHC*DPG]
    wq_d = nc.declare_dram_parameter("wq", [P, NHC * DPG], BF16, isOutput=False)
    wk_d = nc.declare_dram_parameter("wk", [P, NHC * DPG], BF16, isOutput=False)
    wv_d = nc.declare_dram_parameter("wv", [P, NHC * DPG], BF16, isOutput=False)
    bqk_d = nc.declare_dram_parameter("bqk", [P, 4], F32, isOutput=False)
    bvb_d = nc.declare_dram_parameter("bvb", [P, DPG], F32, isOutput=False)
    mk_d = nc.declare_dram_parameter("mk", [P, NKC], F32, isOutput=False)
    out_d = nc.declare_dram_parameter("out", [HPG * VA_W, S], F32, isOutput=True)

    # scores-round emission order (h, pr, kc); pr = pair of q blocks (1024 q)
    rounds = (
        [(h, 0, kc) for h in range(HPG) for kc in range(8)]          # rA
        + [(h, 0, kc) for h in range(HPG) for kc in range(8, 16)]    # rB
        + [(h, 1, kc) for h in range(HPG) for kc in range(16)]       # rC
    )
    NR = len(rounds)  # 128
    ridx = {hpk: r for r, hpk in enumerate(rounds)}
    pq = [None] * NR

    # ctx consumption order: per (pair, i) stream, kc-inner (accumulation)
    ctx_units = [
        (h, pr, i, kc)
        for pr in range(2)
        for h in range(HPG)
        for i in range(2)
        for kc in range(NKC)
    ]

    with tile.TileContext(nc) as tc:
        with (
            tc.tile_pool(name="const", bufs=1) as cpool,
            tc.tile_pool(name="proj_out", bufs=1) as projpool,
            tc.tile_pool(name="xt", bufs=8) as xpool,
            tc.tile_pool(name="xvp", bufs=12) as xvpool,
            tc.tile_pool(name="pexp", bufs=58) as ppool,
            tc.tile_pool(name="outb", bufs=6) as opool,
        ):
            wk_sb = cpool.tile([P, NHC * DPG], BF16)
            wq_sb = cpool.tile([P, NHC * DPG], BF16)
            wv_sb = cpool.tile([P, NHC * DPG], BF16)
            bqk_sb = cpool.tile([P, 4], F32)
            bvb_sb = cpool.tile([P, DPG], F32)
            mk_sb = cpool.tile([P, NKC], F32)

            qT0 = projpool.tile([P, S], BF16)
            qT1 = projpool.tile([P, S], BF16)
            kT0 = projpool.tile([P, S], BF16)
            kT1 = projpool.tile([P, S], BF16)
            va_sb = projpool.tile([P, NKC * HPG * VA_PAD], BF16)

            # wk stripes first (gate the very first matmuls), then misc
            WSTR = (NHC * DPG) // 4  # stripe cols (2 hc chunks) = 512
            for s in range(4):
                nc.sync.dma_start(
                    wk_sb[:, s * WSTR : (s + 1) * WSTR],
                    wk_d[:, s * WSTR : (s + 1) * WSTR],
                )
            nc.sync.dma_start(bqk_sb[:], bqk_d[:])
            nc.sync.dma_start(mk_sb[:], mk_d[:])

            # va zero pad + ones cols (DVE idle early anyway)
            nc.vector.memset(va_sb[:], 0.0)
            for sc in range(NKC):
                for h in range(HPG):
                    oc = (sc * HPG + h) * VA_PAD + HD
                    nc.vector.memset(va_sb[:, oc : oc + 1], 1.0)

            # ---- scores round machinery ----
            state = {"emitted": 0, "acc": 0, "ready": 0}

            def scores_round(r, pool):
                h, pr, kc = rounds[r]
                qT_t = qT0 if h < 2 else qT1
                kT_t = kT0 if h < 2 else kT1
                rows = slice((h % 2) * HD, (h % 2) * HD + HD)
                p2 = ppool.tile([P, 2 * QB], BF16, tag="p", name=f"p{r}")
                s2 = pool.tile([P, 2 * QB], F32, tag="s2", name=f"s2_{r}", bufs=2)
                for i in range(2):
                    qb = pr * 2 + i
                    nc.tensor.matmul(
                        s2[:, i * QB : (i + 1) * QB],
                        kT_t[rows, kc * P : (kc + 1) * P],
                        qT_t[rows, qb * QB : (qb + 1) * QB],
                        start=True,
                        stop=True,
                    )
                nc.scalar.activation(
                    p2[:], s2[:], Exp, bias=mk_sb[:, kc : kc + 1], scale=0.125
                )
                pq[r] = p2

            def pump(pool, cols):
                state["acc"] += cols
                while state["acc"] >= PACE and state["emitted"] < state["ready"]:
                    scores_round(state["emitted"], pool)
                    state["emitted"] += 1
                    state["acc"] = max(state["acc"] - PACE - 2 * QB, 0)

            # ---- q/k projections (W stationary, xT moving) ----
            def proj_pair(x_d, w_sb, bcol, dst0, dst1, pr, psA, spool,
                          extra_dma=None):
                cols0 = pr * 2 * QB
                pp = [
                    psA.tile([P, QB], F32, tag=f"pp{j}", name=f"pp{j}", bufs=1)
                    for j in range(4)
                ]
                for hc in range(NHC):
                    if extra_dma is not None:
                        extra_dma(hc)
                    xt = xpool.tile([P, 2 * QB], BF16, tag="xt", name="xt")
                    nc.sync.dma_start(
                        xt[:, :QB],
                        x_d[hc * P : (hc + 1) * P, cols0 : cols0 + QB],
                    )
                    nc.sync.dma_start(
                        xt[:, QB:],
                        x_d[hc * P : (hc + 1) * P, cols0 + QB : cols0 + 2 * QB],
                    )
                    st = dict(start=(hc == 0), stop=(hc == NHC - 1))
                    w0 = w_sb[:, hc * DPG : hc * DPG + P]
                    w1 = w_sb[:, hc * DPG + P : (hc + 1) * DPG]
                    nc.tensor.matmul(pp[0][:], w0, xt[:, :QB], **st)
                    nc.tensor.matmul(pp[1][:], w0, xt[:, QB:], **st)
                    nc.tensor.matmul(pp[2][:], w1, xt[:, :QB], **st)
                    nc.tensor.matmul(pp[3][:], w1, xt[:, QB:], **st)
                    if spool is not None:
                        pump(spool, 4 * QB)
                for j in range(4):
                    dst = dst0 if j < 2 else dst1
                    bc = bcol + (0 if j < 2 else 1)
                    qb = pr * 2 + (j % 2)
                    nc.vector.tensor_scalar(
                        dst[:, qb * QB : (qb + 1) * QB], pp[j][:],
                        bqk_sb[:, bc : bc + 1], None, ADD,
                    )

            with tc.tile_pool(name="psS", bufs=1, space="PSUM") as psS:
                with tc.tile_pool(name="psA", bufs=1, space="PSUM") as psA:
                    def dma_wq(hc):
                        if hc % 2 == 0:
                            s = hc // 2
                            nc.sync.dma_start(
                                wq_sb[:, s * WSTR : (s + 1) * WSTR],
                                wq_d[:, s * WSTR : (s + 1) * WSTR],
                            )

                    def dma_wv(hc):
                        if hc == 0:
                            nc.sync.dma_start(bvb_sb[:], bvb_d[:])
                        if hc % 2 == 0:
                            s = hc // 2
                            nc.sync.dma_start(
                                wv_sb[:, s * WSTR : (s + 1) * WSTR],
                                wv_d[:, s * WSTR : (s + 1) * WSTR],
                            )

                    proj_pair(xk_d, wk_sb, 2, kT0, kT1, 0, psA, None,
                              extra_dma=dma_wq)
                    proj_pair(xq_d, wq_sb, 0, qT0, qT1, 0, psA, None,
                              extra_dma=dma_wv)
                    state["ready"] = 32
                    proj_pair(xk_d, wk_sb, 2, kT0, kT1, 1, psA, psS)
                    state["ready"] = 64
                    proj_pair(xq_d, wq_sb, 0, qT0, qT1, 1, psA, psS)
                    state["ready"] = NR

                # ---- v projection (x stationary, WvT moving), 4-kc groups,
                # interleaved with paced scores rounds ----
                with tc.tile_pool(name="psV", bufs=1, space="PSUM") as psV:
                    for g in range(NVG):
                        cols0 = g * 4 * P
                        xvt = []
                        for hc in range(NHC):
                            xt = xvpool.tile(
                                [P, 4 * P], BF16, tag="xv", name=f"xv{g}_{hc}"
                            )
                            nc.sync.dma_start(
                                xt[:],
                                xv_d[hc * P : (hc + 1) * P, cols0 : cols0 + 4 * P],
                            )
                            xvt.append(xt)
                        vp = [
                            psV.tile([P, DPG], F32, tag=f"vp{i}",
                                     name=f"vp{i}", bufs=1)
                            for i in range(4)
                        ]
                        for hc in range(NHC):
                            st = dict(start=(hc == 0), stop=(hc == NHC - 1))
                            for i in range(4):
                                nc.tensor.matmul(
                                    vp[i][:],
                                    xvt[hc][:, i * P : (i + 1) * P],
                                    wv_sb[:, hc * DPG : (hc + 1) * DPG],
                                    **st,
                                )
                            pump(psS, 4 * DPG)
                        for i in range(4):
                            kc = g * 4 + i
                            for h in range(HPG):
                                off = (kc * HPG + h) * VA_PAD
                                nc.vector.tensor_tensor(
                                    va_sb[:, off : off + HD],
                                    vp[i][:, h * HD : (h + 1) * HD],
                                    bvb_sb[:, h * HD : (h + 1) * HD],
                                    ADD,
                                )

                # ---- final phase: remaining rounds + i-split ctx streams ----
                with tc.tile_pool(name="psC", bufs=1, space="PSUM") as psC:
                    from collections import deque

                    cq = deque(ctx_units)
                    cur = {}

                    def ctx_unit():
                        h, pr, i, kc = cq.popleft()
                        r = ridx[(h, pr, kc)]
                        key = (h, pr, i)
                        if key not in cur:
                            par = "A" if (pr * HPG * 2 + h * 2 + i) % 2 == 0 else "B"
                            cur[key] = psC.tile(
                                [VA_PAD, QB], F32, tag=f"ct{par}",
                                name=f"ct{h}_{pr}_{i}", bufs=1,
                            )
                        ct = cur[key]
                        off = (kc * HPG + h) * VA_PAD
                        nc.tensor.matmul(
                            ct[:],
                            va_sb[:, off : off + VA_PAD],
                            pq[r][:, i * QB : (i + 1) * QB],
                            start=(kc == 0),
                            stop=(kc == NKC - 1),
                        )
                        if kc == NKC - 1:
                            ct = cur.pop(key)
                            qb = pr * 2 + i
                            ob = opool.tile(
                                [VA_W, QB], F32, tag="ob",
                                name=f"ob{h}_{pr}_{i}",
                            )
                            nc.vector.tensor_copy(ob[:], ct[:VA_W, :])
                            nc.sync.dma_start(
                                out_d[
                                    h * VA_W : (h + 1) * VA_W,
                                    qb * QB : (qb + 1) * QB,
                                ],
                                ob[:],
                            )

                    fill = 0
                    for r in range(state["emitted"], NR):
                        scores_round(r, psS)
                        fill += CTX_FILL
                        while cq and fill >= QB:
                            h2, pr2, i2, kc2 = cq[0]
                            if ridx[(h2, pr2, kc2)] + CTX_LAG <= r + 1:
                                ctx_unit()
                                fill -= QB
                            else:
                                break
                    while cq:
                        ctx_unit()

    nc.compile()
    return nc


def _in_maps(query, key, value, attention_mask, Wq, bq, Wk, bk, Wv, bv):
    import ml_dtypes

    bf16 = ml_dtypes.bfloat16
    q = np.asarray(query, np.float32)
    k = np.asarray(key, np.float32)
    v = np.asarray(value, np.float32)
    m = np.asarray(attention_mask, np.float32)
    Wq = np.asarray(Wq, np.float32)
    Wk = np.asarray(Wk, np.float32)
    Wv = np.asarray(Wv, np.float32)
    bq = np.asarray(bq, np.float32)
    bk = np.asarray(bk, np.float32)
    bv = np.asarray(bv, np.float32)

    def repack_w(W, hs, he):
        # [DPG rows of W] -> SBUF layout [P, NHC*DPG]: chunk hc at cols
        # hc*DPG.., holding W.T[hc*P:(hc+1)*P, :]
        wt = np.ascontiguousarray(W[hs:he, :].T)        # [H, DPG]
        wr = wt.reshape(NHC, P, DPG).transpose(1, 0, 2).reshape(P, NHC * DPG)
        return np.ascontiguousarray(wr).astype(bf16)

    xT = [
        (
            np.ascontiguousarray(q[b].T).astype(bf16),
            np.ascontiguousarray(k[b].T).astype(bf16),
            np.ascontiguousarray(v[b].T).astype(bf16),
        )
        for b in range(B)
    ]
    maps = []
    for c in range(NCORES):
        b, hg = divmod(c, GROUPS)
        hs = hg * DPG
        he = hs + DPG
        bqs, bks = bq[hs:he], bk[hs:he]
        bqk = np.stack([bqs[:P], bqs[P:], bks[:P], bks[P:]], axis=1).astype(
            np.float32
        )
        bvb = np.ascontiguousarray(
            np.broadcast_to(bv[hs:he][None, :], (P, DPG)).astype(np.float32)
        )
        maps.append(
            {
                "xq": xT[b][0],
                "xk": xT[b][1],
                "xv": xT[b][2],
                "wq": repack_w(Wq, hs, he),
                "wk": repack_w(Wk, hs, he),
                "wv": repack_w(Wv, hs, he),
                "bqk": np.ascontiguousarray(bqk),
                "bvb": bvb,
                "mk": np.ascontiguousarray(m[b, 0, 0].reshape(NKC, P).T),
            }
        )
    return maps


def kernel(query, key, value, attention_mask, Wq, bq, Wk, bk, Wv, bv):
    from concourse.bass_utils import run_bass_kernel_spmd

    nc = _build()
    maps = _in_maps(
        query, key, value, attention_mask, Wq, bq, Wk, bk, Wv, bv
    )
    res = run_bass_kernel_spmd(nc, maps, core_ids=list(range(NCORES)))
    out = np.empty((B, S, H), np.float32)
    for c in range(NCORES):
        b, hg = divmod(c, GROUPS)
        o = np.asarray(res.results[c]["out"], np.float32)  # [4*65, S]
        for h in range(HPG):
            blk = o[h * VA_W : (h + 1) * VA_W]
            ctx = blk[:HD] / blk[HD : HD + 1]
            out[b, :, hg * DPG + h * HD : hg * DPG + (h + 1) * HD] = ctx.T
    return out
